# revision 65
# baseline (speedup 1.0000x reference)
"""Trainium2 Bass kernel for the CRA relation module.

Math: the reference computes, per sample,
    phi_x = relu((x@W1+b1)*g1+be1), phi_y likewise,  cat_phi = [phi_x; phi_y]
    A = cat_phi cat_phi^T (symmetric!),  R = [A | A^T] = [A | A]
    W = (cat_phi@W3+b3)@W5a + (R@W4+b4)@W5b + b5
    out = x * W[:196] + y * W[196:]
Because A is symmetric and everything after A is linear into a scalar per
token, the relation pipeline collapses to per-sample matvecs:
    u3 = W3@W5a, u4 = W4@W5b, z = u4[:392]+u4[392:], c0 = b3@W5a+b4@W5b+b5
    s  = u3 + phi_x^T z[:196] + phi_y^T z[196:]          (768-vector)
    out = x*(phi_x@s + c0) + y*(phi_y@s + c0)
The device only computes the two 768x768 "1x1 conv" matmuls (the dominant
cost), one fused multiply-reduce, and one matvec per stream. Everything is
data-parallel over the batch: 16 samples per core on 8 cores.

Device layout: feature-major ("transposed") so the contraction dim (cin)
sits on SBUF partitions. The host packs x into [group, 128, 6*392] where
each 392-column block holds [x_a | x_b] for one cin tile of two samples,
so one DMA per group is fully contiguous.
"""

import ml_dtypes
import numpy as np
from contextlib import ExitStack

BF16_NP = ml_dtypes.bfloat16

import concourse.bass as bass
import concourse.tile as tile
import concourse.mybir as mybir
from concourse.bass_utils import run_bass_kernel_spmd

F32 = mybir.dt.float32
BF16 = mybir.dt.bfloat16
ALU = mybir.AluOpType
ACTF = mybir.ActivationFunctionType


B, N, C = 128, 196, 768
NCORES = 8
S = B // NCORES          # 16 samples per core
G = 2                    # samples per weight pass (moving N = 392 <= 512 fp32)
NG = S // G              # 8 groups per core
DT = C // 128            # 6 feature tiles
W2T = 2 * N              # 392


def build_bass(c0: float) -> bass.Bass:
    nc = bass.Bass()
    xg_d = nc.declare_dram_parameter("xg", [NG, 128, DT * W2T], BF16, isOutput=False)
    yg_d = nc.declare_dram_parameter("yg", [NG, 128, DT * W2T], BF16, isOutput=False)
    w1_d = nc.declare_dram_parameter("w1", [DT, 128, C], BF16, isOutput=False)
    w2_d = nc.declare_dram_parameter("w2", [DT, 128, C], BF16, isOutput=False)
    zb_d = nc.declare_dram_parameter("zb", [128, W2T], BF16, isOutput=False)
    u3_d = nc.declare_dram_parameter("u3", [128, DT], F32, isOutput=False)
    b1_d = nc.declare_dram_parameter("b1", [128, DT], F32, isOutput=False)
    b2_d = nc.declare_dram_parameter("b2", [128, DT], F32, isOutput=False)
    out_d = nc.declare_dram_parameter("out", [S, 128, DT * N], BF16, isOutput=True)

    with tile.TileContext(nc) as tc, ExitStack() as ctx:
        const = ctx.enter_context(tc.tile_pool(name="const", bufs=1))
        xin = ctx.enter_context(tc.tile_pool(name="xin", bufs=4))
        phip = ctx.enter_context(tc.tile_pool(name="phi", bufs=3))
        sp = ctx.enter_context(tc.tile_pool(name="sp", bufs=3))
        op = ctx.enter_context(tc.tile_pool(name="op", bufs=2))
        ps = ctx.enter_context(tc.tile_pool(name="ps", bufs=2, space="PSUM"))

        def dma_group(g):
            xg = xin.tile([128, DT * W2T], BF16, tag="xg", name="xg")
            yg = xin.tile([128, DT * W2T], BF16, tag="yg", name="yg")
            nc.sync.dma_start(out=xg[:], in_=xg_d[g])
            nc.sync.dma_start(out=yg[:], in_=yg_d[g])
            return xg, yg

        # Group 0's inputs first in queue order, then the weights (k-pair
        # interleaved) so the first psum accumulations can start streaming
        # as early as possible.
        xy0 = dma_group(0)
        # Small consts BEFORE the weights: zb gates the very first DVE
        # reduction and b1/b2 gate the first evictions — behind the 4.7MB
        # of weights they'd land ~7us too late. They only delay the weight
        # stream by ~100KB.
        zb = const.tile([128, W2T], BF16, tag="zb")
        nc.sync.dma_start(out=zb[:], in_=zb_d[:, :])
        u3 = const.tile([128, DT], F32, tag="u3")
        nc.sync.dma_start(out=u3[:], in_=u3_d[:, :])
        b1t = const.tile([128, DT], F32, tag="b1")
        nc.sync.dma_start(out=b1t[:], in_=b1_d[:, :])
        b2t = const.tile([128, DT], F32, tag="b2")
        nc.sync.dma_start(out=b2t[:], in_=b2_d[:, :])
        # Weights arrive d-major: tile d holds ALL k-slices for output
        # block d, so block 0's accumulation (and with it the whole
        # eviction -> reduction chain) starts after ~0.4MB instead of
        # after the full 4.7MB weight stream.
        w1_sb, w2_sb = [], []
        for d in range(DT):
            t1 = const.tile([128, C], BF16, tag=f"w1_{d}")
            nc.sync.dma_start(out=t1[:], in_=w1_d[d])
            w1_sb.append(t1)
            t2 = const.tile([128, C], BF16, tag=f"w2_{d}")
            nc.sync.dma_start(out=t2[:], in_=w2_d[d])
            w2_sb.append(t2)
        # Absorb the bias-tile DMA deps into ACT program order now, so the
        # relu evictions later only ever wait on the PE semaphore (the ISA
        # Activation descriptor holds a single sync-wait).
        warm1 = const.tile([128, 1], F32, tag="warm1")
        warm2 = const.tile([128, 1], F32, tag="warm2")
        nc.scalar.activation(warm1[:], b1t[:, 0:1], ACTF.Copy)
        nc.scalar.activation(warm2[:], b2t[:, 0:1], ACTF.Copy)

        def emit_mains(g, xy):
            xg, yg = xy
            # One phi tile PER d-block so consumers' dependencies are exact
            # (a single big tile made every stt wait for all 12 evictions).
            # Layout per d: [a: x(196)|y(196) | b: x(196)|y(196)].
            phd = [phip.tile([128, G * W2T], BF16, tag=f"phd_{d}",
                             name=f"phd_{d}") for d in range(DT)]
            for d in range(DT):
                psx = ps.tile([128, W2T], F32, tag="psx", name="psx", bufs=3)
                psy = ps.tile([128, W2T], F32, tag="psy", name="psy", bufs=3)
                for k in range(DT):
                    nc.tensor.matmul(
                        psx[:], w1_sb[d][:, k * 128:(k + 1) * 128],
                        xg[:, k * W2T:(k + 1) * W2T],
                        start=(k == 0), stop=(k == DT - 1))
                for k in range(DT):
                    nc.tensor.matmul(
                        psy[:], w2_sb[d][:, k * 128:(k + 1) * 128],
                        yg[:, k * W2T:(k + 1) * W2T],
                        start=(k == 0), stop=(k == DT - 1))
                # one eviction per stream: psx [x_a|x_b] scatters to the two
                # samples' x slots of block d (3D strided out AP)
                phv = phd[d][:].rearrange("p (i s t) -> p i s t", i=G, s=2)
                nc.scalar.activation(phv[:, :, 0, :],
                                     psx[:].rearrange("p (i t) -> p i t", i=G),
                                     ACTF.Relu, bias=b1t[:, d:d + 1])
                nc.scalar.activation(phv[:, :, 1, :],
                                     psy[:].rearrange("p (i t) -> p i t", i=G),
                                     ACTF.Relu, bias=b2t[:, d:d + 1])
            return xg, yg, phd

        def emit_head(g, xg, yg, phd):
            # s = u3 + phi_x^T zx + phi_y^T zy, then wxy = phi @ s via a
            # stride-0 broadcast lhsT. DVE does the reductions, GPSIMD the
            # tiny s adds, PE the matvec.
            psws = []
            for i in range(G):
                t_sb = sp.tile([128, DT], F32, tag=f"t_{i}", name=f"t_{i}")
                s_sb = sp.tile([128, DT], BF16, tag=f"s_{i}", name=f"s_{i}")
                psw = ps.tile([128, W2T], F32, tag="psw", name="psw", bufs=2)
                # Scratch writes go into psw: its matmul accumulation below
                # resets the bank (start=True), and PSUM writes don't fight
                # ACT/GPSIMD/DMA for SBUF ports.
                for d in range(DT):
                    nc.vector.scalar_tensor_tensor(
                        out=psw[:], in0=phd[d][:, i * W2T:(i + 1) * W2T],
                        scalar=1.0, in1=zb[:],
                        op0=ALU.mult, op1=ALU.mult,
                        accum_out=t_sb[:, d:d + 1])
                nc.gpsimd.tensor_tensor(s_sb[:], t_sb[:], u3[:], ALU.add)
                for d in range(DT):
                    nc.tensor.matmul(
                        psw[:], s_sb[:, d:d + 1].broadcast_to([128, 128]),
                        phd[d][:, i * W2T:(i + 1) * W2T],
                        start=(d == 0), stop=(d == DT - 1))
                # evict with +c0 folded in
                wxy = sp.tile([128, W2T], BF16, tag=f"wxy_{i}", name=f"wxy_{i}")
                nc.scalar.activation(wxy[:], psw[:], ACTF.Copy, bias=c0)
                psws.append(wxy)
            return psws

        def emit_finish(g, xg, yg, psws, split_y=False):
            for i in range(G):
                wxy = psws[i]
                # out^T = x^T * wx + y^T * wy — combine add on GPSIMD
                # (SBUF-only there). split_y puts the y multiply on GPSIMD
                # too — only worth it in the drain, when DVE is the sole
                # straggler and SBUF contention no longer matters.
                osb = op.tile([128, DT * N], BF16, tag=f"osb_{i}", name=f"osb_{i}")
                tmp = op.tile([128, DT * N], BF16, tag=f"tmp_{i}", name=f"tmp_{i}")
                xv = xg[:].rearrange("p (d q) -> p d q", d=DT)[:, :, i * N:(i + 1) * N]
                yv = yg[:].rearrange("p (d q) -> p d q", d=DT)[:, :, i * N:(i + 1) * N]
                ov = osb[:].rearrange("p (d t) -> p d t", d=DT)
                tv = tmp[:].rearrange("p (d t) -> p d t", d=DT)
                wx = wxy[:, 0:N].unsqueeze(1).broadcast_to([128, DT, N])
                wy = wxy[:, N:W2T].unsqueeze(1).broadcast_to([128, DT, N])
                y_eng = nc.gpsimd if split_y else nc.vector
                y_eng.tensor_tensor(tv, wy, yv, ALU.mult)
                nc.vector.tensor_tensor(ov, wx, xv, ALU.mult)
                nc.gpsimd.tensor_tensor(osb[:], osb[:], tmp[:], ALU.add)
                nc.sync.dma_start(out=out_d[G * g + i], in_=osb[:])

        # 3-stage software pipeline: PE runs group g's dense matmuls while
        # group g-1's reduction chain feeds its matvec and group g-2's
        # final elementwise combine drains. Keeping heads ahead of finishes
        # in DVE/GPSIMD program order means the PE matvec never waits on
        # the (slower) elementwise stage.
        mains = {}
        heads = {}
        xy = xy0
        for g in range(NG):
            mains[g] = emit_mains(g, xy)
            if g + 1 < NG:
                xy = dma_group(g + 1)
            if g >= 1:
                heads[g - 1] = emit_head(g - 1, *mains[g - 1])
            # Hold back the last in-loop finish (g-2 == NG-3): emitting it
            # here would wedge its finals between stts(NG-2) and stts(NG-1)
            # in DVE program order, delaying the last psw matvecs (and so
            # the PE tail) by the finals' duration.
            if 2 <= g and g - 2 < NG - 3:
                emit_finish(g - 2, mains[g - 2][0], mains[g - 2][1], heads[g - 2])
        heads[NG - 1] = emit_head(NG - 1, *mains[NG - 1])
        emit_finish(NG - 3, mains[NG - 3][0], mains[NG - 3][1], heads[NG - 3])
        emit_finish(NG - 2, mains[NG - 2][0], mains[NG - 2][1], heads[NG - 2],
                    split_y=True)
        emit_finish(NG - 1, mains[NG - 1][0], mains[NG - 1][1], heads[NG - 1],
                    split_y=True)

    _split_multi_waits(nc)
    return nc


def _split_multi_waits(nc):
    """This walrus build accepts at most ONE sync-wait command per TPB
    instruction; the Tile scheduler happily emits several. Hoist all but the
    last wait of each instruction onto same-engine EventSemaphore ops placed
    immediately before it (engine program order is the within-block
    subsequence, so this preserves semantics)."""
    import json
    data = json.loads(nc.to_json_bytes())
    n = 0
    for fn in data["functions"]:
        for blk in fn["blocks"]:
            out = []
            for inst in blk["instructions"]:
                si = inst.get("sync_info")
                ow = (si or {}).get("on_wait") or []
                if len(ow) > 1:
                    for w in ow[:-1]:
                        n += 1
                        out.append({
                            "name": f"eswait_{n}",
                            "opcode": "EventSemaphore",
                            "engine": inst["engine"],
                            "ins": [],
                            "outs": [],
                            "sync_info": {"on_wait": [w], "on_update": []},
                        })
                    si["on_wait"] = [ow[-1]]
                out.append(inst)
            blk["instructions"] = out
    nc.m = mybir.module_from_json_bytes(json.dumps(data).encode())
    return nc


def prep_host(inputs: dict):
    x = np.ascontiguousarray(np.asarray(inputs["x"], dtype=np.float32))
    y = np.ascontiguousarray(np.asarray(inputs["y"], dtype=np.float32))
    W1 = np.asarray(inputs["W1"], dtype=np.float32)
    W2 = np.asarray(inputs["W2"], dtype=np.float32)
    g1 = np.asarray(inputs["g1"], dtype=np.float32)
    g2 = np.asarray(inputs["g2"], dtype=np.float32)
    b1 = np.asarray(inputs["b1"], dtype=np.float32)
    b2 = np.asarray(inputs["b2"], dtype=np.float32)
    be1 = np.asarray(inputs["be1"], dtype=np.float32)
    be2 = np.asarray(inputs["be2"], dtype=np.float32)
    W3 = np.asarray(inputs["W3"], dtype=np.float32)
    b3 = np.asarray(inputs["b3"], dtype=np.float32)
    W4 = np.asarray(inputs["W4"], dtype=np.float32)
    b4 = np.asarray(inputs["b4"], dtype=np.float32)
    W5 = np.asarray(inputs["W5"], dtype=np.float32)
    b5 = np.asarray(inputs["b5"], dtype=np.float32)

    def pack_w(w):
        # [C, C] -> [DT, 128, C]: block d holds [k-part p, k*128+j] =
        # w[k*128+p, d*128+j], the d-major SBUF image
        return np.ascontiguousarray(
            w.astype(BF16_NP).reshape(DT, 128, DT, 128)
            .transpose(2, 1, 0, 3).reshape(DT, 128, C))

    W1p = pack_w(W1 * g1[None, :])
    W2p = pack_w(W2 * g2[None, :])
    b1p = b1 * g1 + be1
    b2p = b2 * g2 + be2
    W5a, W5b = W5[:C, 0], W5[C:, 0]
    u3 = (W3 @ W5a).astype(np.float32)
    u4 = (W4 @ W5b).astype(np.float32)
    z = (u4[:2 * N] + u4[2 * N:]).astype(np.float32)
    c0 = float(b3 @ W5a + b4 @ W5b + b5[0])

    # [B,N,C] -> per-core groups [M, NG, 128, DT*392] with [x_a|x_b] 392-blocks
    def pack(a):
        at = a.transpose(0, 2, 1).reshape(NCORES, S, DT, 128, N)
        pair = at.reshape(NCORES, NG, G, DT, 128, N)
        gg = np.concatenate([pair[:, :, 0], pair[:, :, 1]], axis=-1)  # [M,NG,DT,128,392]
        return np.ascontiguousarray(
            gg.transpose(0, 1, 3, 2, 4).reshape(NCORES, NG, 128, DT * W2T)
            .astype(BF16_NP))

    XG, YG = pack(x), pack(y)
    zb = np.ascontiguousarray(np.broadcast_to(z[None, :], (128, W2T)).astype(BF16_NP))
    u3t = np.ascontiguousarray(u3.reshape(DT, 128).T)
    b1t = np.ascontiguousarray(b1p.reshape(DT, 128).T)
    b2t = np.ascontiguousarray(b2p.reshape(DT, 128).T)

    in_maps = []
    for cidx in range(NCORES):
        in_maps.append({
            "xg": XG[cidx], "yg": YG[cidx], "w1": W1p, "w2": W2p,
            "zb": zb, "u3": u3t, "b1": b1t, "b2": b2t,
        })
    return in_maps, c0, x, y


def unpack_out(results) -> np.ndarray:
    outs = []
    for cidx in range(NCORES):
        o = np.asarray(results[cidx]["out"]).astype(np.float32)  # [S, 128, DT*N]
        o = o.reshape(S, 128, DT, N).transpose(0, 2, 1, 3).reshape(S, C, N)
        outs.append(o.transpose(0, 2, 1))     # [S, N, C]
    return np.ascontiguousarray(np.concatenate(outs, axis=0))


def kernel(**inputs) -> np.ndarray:
    in_maps, c0, _, _ = prep_host(inputs)
    nc = build_bass(c0)
    res = run_bass_kernel_spmd(nc, in_maps, list(range(NCORES)))
    return unpack_out(res.results)

